# revision 13
# baseline (speedup 1.0000x reference)
"""MoE ConditionalFeedForward kernel for 8 trn2 NeuronCores.

Strategy: expert parallelism with 2-way intermediate (I) splitting for load
balance. The 8 experts are ranked by routed-token count and paired
heaviest-with-lightest into 4 groups; the two cores of group g each own HALF
the I-rows (22 of 44 128-row blocks) of BOTH experts in the group and process
ALL tokens routed to them. Each core therefore streams exactly 44 block-rows
of w1/w3/w2 (the same HBM traffic as one full expert) while its matmul column
count is bounded by max-heavy + max-light counts instead of the global max.
Token (t, slot) pairs whose two slots hit the same expert are deduplicated
(computed once, scattered twice).

Per core, slice s in {heavy, light} with capacity C_s:
  phase 1: hT[i, c] = silu(sum_d w1T[d,i] xT[d,c]) * (sum_d w3T[d,i] xT[d,c])
           for the 22 owned i-blocks (PE matmuls, d on partitions)
  phase 2: yT[d, c] = sum_{i in owned} hT[i, c] * w2[i, d]  (partial sum)
The two partial yT of an expert are summed on the host (f32) and scattered.

All weights/activations stream as bf16; PSUM accumulation is f32.
"""

import numpy as np
import ml_dtypes

BF16 = ml_dtypes.bfloat16

# Problem dims (hardcoded per contract; kernel.py must be self-contained).
T, A, E, D, I = 1024, 2, 8, 2048, 5632
N_CORES = 8
DB = D // 128          # 16 d-chunks (phase-1 contraction)
IB = I // 128          # 44 i-blocks total per expert
HB = IB // 2           # 22 i-blocks per core slice
NPASS = 8              # phase-2 passes over D
W = D // NPASS         # 256 output columns per phase-2 pass
NDC = W // 128         # 2 128-col d-blocks per pass
GSZ = 8                # w2 i-blocks per DMA group

_BUILD_CACHE = {}


def _pad4(n):
    return max(4, -(-int(n) // 4) * 4)


def _build(C1, C2):
    """Build + compile the per-core Bass program for slice capacities C1, C2."""
    key = (C1, C2)
    if key in _BUILD_CACHE:
        return _BUILD_CACHE[key]

    import concourse.mybir as mybir
    import concourse.tile as tile
    from concourse import bacc

    dt = mybir.dt
    WDT = dt.bfloat16
    F32 = dt.float32

    assert C1 <= 512 and C2 <= C1

    nc = bacc.Bacc("TRN2", target_bir_lowering=False, debug=False,
                   num_devices=N_CORES)

    xa_t = nc.dram_tensor("xga", [128, DB * C1], WDT, kind="ExternalInput").ap()
    xb_t = nc.dram_tensor("xgb", [128, DB * C2], WDT, kind="ExternalInput").ap()
    w1a_t = nc.dram_tensor("w1a", [128, HB * DB * 128], WDT,
                           kind="ExternalInput").ap()
    w3a_t = nc.dram_tensor("w3a", [128, HB * DB * 128], WDT,
                           kind="ExternalInput").ap()
    w1b_t = nc.dram_tensor("w1b", [128, HB * DB * 128], WDT,
                           kind="ExternalInput").ap()
    w3b_t = nc.dram_tensor("w3b", [128, HB * DB * 128], WDT,
                           kind="ExternalInput").ap()
    w2a_t = nc.dram_tensor("w2a", [NPASS, 128, HB * W], WDT,
                           kind="ExternalInput").ap()
    w2b_t = nc.dram_tensor("w2b", [NPASS, 128, HB * W], WDT,
                           kind="ExternalInput").ap()
    # outputs are y transposed ([D, C]) partial sums; host adds + untransposes.
    ya_t = nc.dram_tensor("yta", [D, C1], F32, kind="ExternalOutput").ap()
    yb_t = nc.dram_tensor("ytb", [D, C2], F32, kind="ExternalOutput").ap()

    slices = [(C1, xa_t, w1a_t, w3a_t, w2a_t, ya_t),
              (C2, xb_t, w1b_t, w3b_t, w2b_t, yb_t)]

    with tile.TileContext(nc) as tc:
        with (
            tc.tile_pool(name="xpool", bufs=1) as xpool,
            tc.tile_pool(name="w1pool", bufs=5) as w1pool,
            tc.tile_pool(name="w3pool", bufs=5) as w3pool,
            # deep w2 prefetch: fills the DMA-idle trough at each phase-1
            # tail (w1/w3 fully delivered ~20us before phase 1 ends) so
            # phase 2 never starves at pass boundaries.
            tc.tile_pool(name="w2pool", bufs=16) as w2pool,
            tc.tile_pool(name="hpool", bufs=1) as hpool,
            tc.tile_pool(name="spool", bufs=2) as spool,
            tc.tile_pool(name="opool", bufs=4) as opool,
            tc.tile_pool(name="ps", bufs=2, space="PSUM") as ps,
        ):
            xg = {}
            for s, (C, x_t, *_r) in enumerate(slices):
                xg[s] = xpool.tile([128, DB * C], WDT, tag=f"x{s}",
                                   name=f"xg{s}")

            def issue_w13(s, gi, nb, b0):
                """Create + DMA one phase-1 weight group for slice s."""
                w1_t, w3_t = slices[s][2], slices[s][3]
                wt1 = w1pool.tile([128, 2 * DB * 128], WDT, tag="w1",
                                  name="wt1")
                wt3 = w3pool.tile([128, 2 * DB * 128], WDT, tag="w3",
                                  name="wt3")
                span = nb * DB * 128
                lo = b0 * DB * 128
                if s == 0 and gi == 0:
                    # startup-critical ordering: w1 g0, then x (the first
                    # chain gates on both), then w3 g0 right behind.
                    C = slices[0][0]
                    x_t = slices[0][1]
                    nc.sync.dma_start(wt1[:, :span], w1_t[:, lo:lo + span])
                    nc.sync.dma_start(xg[0][:, :4 * C], x_t[:, :4 * C])
                    nc.sync.dma_start(xg[0][:, 4 * C:], x_t[:, 4 * C:])
                    nc.sync.dma_start(wt3[:, :span], w3_t[:, lo:lo + span])
                else:
                    nc.sync.dma_start(wt1[:, :span], w1_t[:, lo:lo + span])
                    nc.sync.dma_start(wt3[:, :span], w3_t[:, lo:lo + span])
                if s == 0 and gi == 9:
                    # slice-1 x, issued past the startup ramp so it
                    # streams during slice-0 compute (needed much later).
                    C2_, xb = slices[1][0], slices[1][1]
                    for q0 in range(0, DB * C2_, 8 * C2_):
                        q1 = min(q0 + 8 * C2_, DB * C2_)
                        nc.sync.dma_start(xg[1][:, q0:q1], xb[:, q0:q1])
                return wt1, wt3

            def groups_of(s):
                return ([1, 1] + [2] * 10) if s == 0 else [2] * 11

            def emit_phase1(s, h, pre_issued):
                """Phase 1 matmuls + silu for slice s. pre_issued: list of
                (wt1, wt3, nb, b0) groups already DMA'd during the previous
                slice's phase 2."""
                C = slices[s][0]
                glist = []
                b0 = 0
                for gi, nb in enumerate(groups_of(s)):
                    if gi < len(pre_issued):
                        glist.append(pre_issued[gi])
                    else:
                        wt1, wt3 = issue_w13(s, gi, nb, b0)
                        glist.append((wt1, wt3, nb, b0))
                    b0 += nb
                for wt1, wt3, nb, b0 in glist:
                    for sb in range(nb):
                        b = b0 + sb
                        ps1 = ps.tile([128, C1], F32, tag="ps1")
                        ps3 = ps.tile([128, C1], F32, tag="ps3")
                        for do in range(DB):
                            lo = (sb * DB + do) * 128
                            nc.tensor.matmul(
                                ps1[:, :C], wt1[:, lo:lo + 128],
                                xg[s][:, do * C:(do + 1) * C],
                                start=(do == 0), stop=(do == DB - 1))
                        for do in range(DB):
                            lo = (sb * DB + do) * 128
                            nc.tensor.matmul(
                                ps3[:, :C], wt3[:, lo:lo + 128],
                                xg[s][:, do * C:(do + 1) * C],
                                start=(do == 0), stop=(do == DB - 1))
                        sig = spool.tile([128, C1], F32, tag="sig")
                        nc.scalar.activation(
                            sig[:, :C], ps1[:, :C],
                            mybir.ActivationFunctionType.Sigmoid)
                        m1 = spool.tile([128, C1], F32, tag="m1")
                        nc.vector.tensor_mul(m1[:, :C], sig[:, :C], ps3[:, :C])
                        nc.vector.tensor_mul(
                            h[:, b * C:(b + 1) * C], m1[:, :C], ps1[:, :C])

            def emit_phase2(s, h, next_slice):
                """Phase 2 for slice s. If next_slice is set, weave the next
                slice's first phase-1 weight-group DMAs between early w2
                groups (sync issues them while w2 prefetch is still in its
                immediate-fire window) and return them for emit_phase1."""
                C, _x, _w1, _w3, w2_t, y_t = slices[s]
                w2groups = [(g0, min(GSZ, HB - g0)) for g0 in range(0, HB, GSZ)]
                pre = []
                gctr = 0
                for ph in range(NPASS):
                    po = {}
                    for dc in range(NDC):
                        po[dc] = ps.tile([128, C1], F32, tag=f"y{dc}",
                                         name=f"po{dc}")
                    wts = []
                    for g0, nb in w2groups:
                        wt2 = w2pool.tile([128, GSZ * W], WDT, tag="w2",
                                          name="wt2")
                        nc.sync.dma_start(wt2[:, :nb * W],
                                          w2_t[ph][:, g0 * W:(g0 + nb) * W])
                        wts.append((wt2, g0, nb))
                        gctr += 1
                        if (next_slice is not None and gctr % 2 == 0
                                and len(pre) < 4):
                            gi = len(pre)
                            nb_n = groups_of(next_slice)[gi]
                            b0_n = sum(groups_of(next_slice)[:gi])
                            wt1n, wt3n = issue_w13(next_slice, gi, nb_n, b0_n)
                            pre.append((wt1n, wt3n, nb_n, b0_n))
                    # un-interleaved dc chains: dc0's drain (copy + output
                    # DMA) hides under dc1's matmul chain.
                    for dc in range(NDC):
                        for wt2, g0, nb in wts:
                            for sb in range(nb):
                                b = g0 + sb
                                lo = sb * W + dc * 128
                                nc.tensor.matmul(
                                    po[dc][:, :C],
                                    wt2[:, lo:lo + 128],
                                    h[:, b * C:(b + 1) * C],
                                    start=(b == 0), stop=(b == HB - 1))
                        ot = opool.tile([128, C1], F32, tag="ot")
                        nc.vector.tensor_copy(ot[:, :C], po[dc][:, :C])
                        # final pass drains on the kernel's critical tail:
                        # issue from the otherwise-idle gpsimd engine so the
                        # two final issues don't serialize on scalar.
                        eng = nc.gpsimd if ph == NPASS - 1 else nc.scalar
                        eng.dma_start(
                            y_t[ph * W + dc * 128:ph * W + dc * 128 + 128, :],
                            ot[:, :C])
                return pre

            h0 = hpool.tile([128, HB * C1], WDT, tag="h0")
            h1 = hpool.tile([128, HB * C2], WDT, tag="h1")
            emit_phase1(0, h0, [])
            pre_b = emit_phase2(0, h0, next_slice=1)
            emit_phase1(1, h1, pre_b)
            emit_phase2(1, h1, next_slice=None)

    nc.compile()
    _BUILD_CACHE[key] = nc
    return nc


def _pack13(wh):
    """[2816, 2048] w1/w3 half -> phase-1 layout [128, HB*DB*128]:
    col = (b*DB + do)*128 + i_in, partition = d_in."""
    return np.ascontiguousarray(
        wh.reshape(HB, 128, DB, 128).transpose(3, 0, 2, 1)
    ).reshape(128, HB * DB * 128)


def _pack2(wh):
    """[2816, 2048] w2 half -> phase-2 layout [NPASS, 128, HB*W]:
    per pass, col = b*W + j, partition = i_in."""
    return np.ascontiguousarray(
        wh.reshape(HB, 128, NPASS, W).transpose(2, 1, 0, 3)
    ).reshape(NPASS, 128, HB * W)


def _packx(x_bf, tokens, C):
    """Gather token rows of x (bf16) and lay out as [128, DB*C]:
    col = do*C + c, partition = d_in."""
    xp = np.zeros((C, D), BF16)
    xp[:len(tokens)] = x_bf[tokens]
    return np.ascontiguousarray(
        xp.reshape(C, DB, 128).transpose(2, 1, 0)
    ).reshape(128, DB * C)


def _prepare(inputs):
    """Host routing + packing. Returns (nc, in_maps, scatter_info)."""
    x = np.asarray(inputs["x"])
    idx = np.asarray(inputs["expert_indices"])
    w1 = np.asarray(inputs["w1"])
    w2 = np.asarray(inputs["w2"])
    w3 = np.asarray(inputs["w3"])

    t_n, a_n = idx.shape

    # ---- dedup + routing ----
    tt = np.repeat(np.arange(t_n), a_n)
    ee = idx.reshape(-1).astype(np.int64)
    keys = tt * E + ee
    uniq = np.unique(keys)                        # sorted (t, e) pairs
    ue = uniq % E
    ut = uniq // E
    order = np.argsort(ue, kind="stable")         # grouped by expert
    counts = np.bincount(ue, minlength=E)
    starts = np.concatenate([[0], np.cumsum(counts)])
    # concat-layout row of each unique pair, and the gather map for scatter
    col = np.empty(len(uniq), np.int64)
    col[order] = np.arange(len(uniq)) - starts[ue[order]]
    concat_row = starts[ue] + col
    gather_rows = concat_row[np.searchsorted(uniq, keys)]   # [T*A]

    # ---- heavy/light pairing ----
    rank = np.argsort(-counts, kind="stable")
    pairs = [(int(rank[i]), int(rank[7 - i])) for i in range(4)]
    C1 = _pad4(counts[rank[0]])
    C2 = _pad4(counts[rank[4]])
    tokens_of = {
        int(e): ut[order[starts[e]:starts[e] + counts[e]]] for e in range(E)
    }

    nc = _build(C1, C2)

    x_bf = x.astype(BF16)
    w1_bf = {}
    in_maps = [dict() for _ in range(N_CORES)]
    for g, (he, le) in enumerate(pairs):
        xa = _packx(x_bf, tokens_of[he], C1)
        xb = _packx(x_bf, tokens_of[le], C2)
        for half in range(2):
            c = 2 * g + half
            r0, r1 = half * (I // 2), (half + 1) * (I // 2)
            in_maps[c]["xga"] = xa
            in_maps[c]["xgb"] = xb
            in_maps[c]["w1a"] = _pack13(w1[he][r0:r1].astype(BF16))
            in_maps[c]["w3a"] = _pack13(w3[he][r0:r1].astype(BF16))
            in_maps[c]["w2a"] = _pack2(w2[he][r0:r1].astype(BF16))
            in_maps[c]["w1b"] = _pack13(w1[le][r0:r1].astype(BF16))
            in_maps[c]["w3b"] = _pack13(w3[le][r0:r1].astype(BF16))
            in_maps[c]["w2b"] = _pack2(w2[le][r0:r1].astype(BF16))

    scatter_info = (t_n, a_n, pairs, counts, starts, gather_rows, len(uniq))
    return nc, in_maps, scatter_info


def _scatter(results, scatter_info):
    t_n, a_n, pairs, counts, starts, gather_rows, n_uniq = scatter_info
    yc = np.empty((n_uniq, D), np.float32)
    for g, (he, le) in enumerate(pairs):
        ya = results[2 * g]["yta"] + results[2 * g + 1]["yta"]   # [D, C1]
        yb = results[2 * g]["ytb"] + results[2 * g + 1]["ytb"]   # [D, C2]
        yc[starts[he]:starts[he] + counts[he]] = ya[:, :counts[he]].T
        yc[starts[le]:starts[le] + counts[le]] = yb[:, :counts[le]].T
    return yc[gather_rows].reshape(t_n, a_n, D)


def kernel(**inputs):
    from concourse.bass_utils import run_bass_kernel_spmd

    nc, in_maps, scatter_info = _prepare(inputs)
    res = run_bass_kernel_spmd(nc, in_maps, core_ids=list(range(N_CORES)))
    return _scatter(res.results, scatter_info)


# revision 14
# speedup vs baseline: 1.0030x; 1.0030x over previous
"""MoE ConditionalFeedForward kernel for 8 trn2 NeuronCores.

Strategy: expert parallelism with 2-way intermediate (I) splitting for load
balance. The 8 experts are ranked by routed-token count and paired
heaviest-with-lightest into 4 groups; the two cores of group g each own HALF
the I-rows (22 of 44 128-row blocks) of BOTH experts in the group and process
ALL tokens routed to them. Each core therefore streams exactly 44 block-rows
of w1/w3/w2 (the same HBM traffic as one full expert) while its matmul column
count is bounded by max-heavy + max-light counts instead of the global max.
Token (t, slot) pairs whose two slots hit the same expert are deduplicated
(computed once, scattered twice).

Per core, slice s in {heavy, light} with capacity C_s:
  phase 1: hT[i, c] = silu(sum_d w1T[d,i] xT[d,c]) * (sum_d w3T[d,i] xT[d,c])
           for the 22 owned i-blocks (PE matmuls, d on partitions)
  phase 2: yT[d, c] = sum_{i in owned} hT[i, c] * w2[i, d]  (partial sum)
The two partial yT of an expert are summed on the host (f32) and scattered.

All weights/activations stream as bf16; PSUM accumulation is f32.
"""

import numpy as np
import ml_dtypes

BF16 = ml_dtypes.bfloat16

# Problem dims (hardcoded per contract; kernel.py must be self-contained).
T, A, E, D, I = 1024, 2, 8, 2048, 5632
N_CORES = 8
DB = D // 128          # 16 d-chunks (phase-1 contraction)
IB = I // 128          # 44 i-blocks total per expert
HB = IB // 2           # 22 i-blocks per core slice
NPASS = 8              # phase-2 passes over D
W = D // NPASS         # 256 output columns per phase-2 pass
NDC = W // 128         # 2 128-col d-blocks per pass
GSZ = 8                # w2 i-blocks per DMA group

_BUILD_CACHE = {}


def _pad4(n):
    return max(4, -(-int(n) // 4) * 4)


def _build(C1, C2):
    """Build + compile the per-core Bass program for slice capacities C1, C2."""
    key = (C1, C2)
    if key in _BUILD_CACHE:
        return _BUILD_CACHE[key]

    import concourse.mybir as mybir
    import concourse.tile as tile
    from concourse import bacc

    dt = mybir.dt
    WDT = dt.bfloat16
    F32 = dt.float32

    assert C1 <= 512 and C2 <= C1

    nc = bacc.Bacc("TRN2", target_bir_lowering=False, debug=False,
                   num_devices=N_CORES)

    xa_t = nc.dram_tensor("xga", [128, DB * C1], WDT, kind="ExternalInput").ap()
    xb_t = nc.dram_tensor("xgb", [128, DB * C2], WDT, kind="ExternalInput").ap()
    w1a_t = nc.dram_tensor("w1a", [128, HB * DB * 128], WDT,
                           kind="ExternalInput").ap()
    w3a_t = nc.dram_tensor("w3a", [128, HB * DB * 128], WDT,
                           kind="ExternalInput").ap()
    w1b_t = nc.dram_tensor("w1b", [128, HB * DB * 128], WDT,
                           kind="ExternalInput").ap()
    w3b_t = nc.dram_tensor("w3b", [128, HB * DB * 128], WDT,
                           kind="ExternalInput").ap()
    w2a_t = nc.dram_tensor("w2a", [NPASS, 128, HB * W], WDT,
                           kind="ExternalInput").ap()
    w2b_t = nc.dram_tensor("w2b", [NPASS, 128, HB * W], WDT,
                           kind="ExternalInput").ap()
    # outputs are y transposed ([D, C]) partial sums; host adds + untransposes.
    ya_t = nc.dram_tensor("yta", [D, C1], F32, kind="ExternalOutput").ap()
    yb_t = nc.dram_tensor("ytb", [D, C2], F32, kind="ExternalOutput").ap()

    slices = [(C1, xa_t, w1a_t, w3a_t, w2a_t, ya_t),
              (C2, xb_t, w1b_t, w3b_t, w2b_t, yb_t)]

    with tile.TileContext(nc) as tc:
        with (
            tc.tile_pool(name="xpool", bufs=1) as xpool,
            tc.tile_pool(name="w1pool", bufs=5) as w1pool,
            tc.tile_pool(name="w3pool", bufs=5) as w3pool,
            # deep w2 prefetch: fills the DMA-idle trough at each phase-1
            # tail (w1/w3 fully delivered ~20us before phase 1 ends) so
            # phase 2 never starves at pass boundaries.
            tc.tile_pool(name="w2pool", bufs=16) as w2pool,
            tc.tile_pool(name="hpool", bufs=1) as hpool,
            tc.tile_pool(name="spool", bufs=2) as spool,
            tc.tile_pool(name="opool", bufs=4) as opool,
            tc.tile_pool(name="ps", bufs=2, space="PSUM") as ps,
        ):
            xg = {}
            for s, (C, x_t, *_r) in enumerate(slices):
                xg[s] = xpool.tile([128, DB * C], WDT, tag=f"x{s}",
                                   name=f"xg{s}")

            def issue_w13(s, gi, nb, b0):
                """Create + DMA one phase-1 weight group for slice s."""
                w1_t, w3_t = slices[s][2], slices[s][3]
                wt1 = w1pool.tile([128, 2 * DB * 128], WDT, tag="w1",
                                  name="wt1")
                wt3 = w3pool.tile([128, 2 * DB * 128], WDT, tag="w3",
                                  name="wt3")
                span = nb * DB * 128
                lo = b0 * DB * 128
                if s == 0 and gi == 0:
                    # startup-critical ordering: w1 g0, then x (the first
                    # chain gates on both), then w3 g0 right behind.
                    C = slices[0][0]
                    x_t = slices[0][1]
                    nc.sync.dma_start(wt1[:, :span], w1_t[:, lo:lo + span])
                    nc.sync.dma_start(xg[0][:, :4 * C], x_t[:, :4 * C])
                    nc.sync.dma_start(xg[0][:, 4 * C:], x_t[:, 4 * C:])
                    nc.sync.dma_start(wt3[:, :span], w3_t[:, lo:lo + span])
                else:
                    nc.sync.dma_start(wt1[:, :span], w1_t[:, lo:lo + span])
                    nc.sync.dma_start(wt3[:, :span], w3_t[:, lo:lo + span])
                if s == 0 and gi == 9:
                    # slice-1 x, issued past the startup ramp so it
                    # streams during slice-0 compute (needed much later).
                    C2_, xb = slices[1][0], slices[1][1]
                    for q0 in range(0, DB * C2_, 8 * C2_):
                        q1 = min(q0 + 8 * C2_, DB * C2_)
                        nc.sync.dma_start(xg[1][:, q0:q1], xb[:, q0:q1])
                return wt1, wt3

            def groups_of(s):
                return ([1, 1] + [2] * 10) if s == 0 else [2] * 11

            def emit_phase1(s, h, pre_issued):
                """Phase 1 matmuls + silu for slice s. pre_issued: list of
                (wt1, wt3, nb, b0) groups already DMA'd during the previous
                slice's phase 2."""
                C = slices[s][0]
                glist = []
                b0 = 0
                for gi, nb in enumerate(groups_of(s)):
                    if gi < len(pre_issued):
                        glist.append(pre_issued[gi])
                    else:
                        wt1, wt3 = issue_w13(s, gi, nb, b0)
                        glist.append((wt1, wt3, nb, b0))
                    b0 += nb
                for wt1, wt3, nb, b0 in glist:
                    for sb in range(nb):
                        b = b0 + sb
                        ps1 = ps.tile([128, C1], F32, tag="ps1")
                        ps3 = ps.tile([128, C1], F32, tag="ps3")
                        for do in range(DB):
                            lo = (sb * DB + do) * 128
                            nc.tensor.matmul(
                                ps1[:, :C], wt1[:, lo:lo + 128],
                                xg[s][:, do * C:(do + 1) * C],
                                start=(do == 0), stop=(do == DB - 1))
                        for do in range(DB):
                            lo = (sb * DB + do) * 128
                            nc.tensor.matmul(
                                ps3[:, :C], wt3[:, lo:lo + 128],
                                xg[s][:, do * C:(do + 1) * C],
                                start=(do == 0), stop=(do == DB - 1))
                        sig = spool.tile([128, C1], F32, tag="sig")
                        nc.scalar.activation(
                            sig[:, :C], ps1[:, :C],
                            mybir.ActivationFunctionType.Sigmoid)
                        m1 = spool.tile([128, C1], F32, tag="m1")
                        nc.vector.tensor_mul(m1[:, :C], sig[:, :C], ps3[:, :C])
                        nc.vector.tensor_mul(
                            h[:, b * C:(b + 1) * C], m1[:, :C], ps1[:, :C])

            def emit_phase2(s, h, next_slice):
                """Phase 2 for slice s. If next_slice is set, weave the next
                slice's first phase-1 weight-group DMAs between early w2
                groups (sync issues them while w2 prefetch is still in its
                immediate-fire window) and return them for emit_phase1."""
                C, _x, _w1, _w3, w2_t, y_t = slices[s]
                w2groups = [(g0, min(GSZ, HB - g0)) for g0 in range(0, HB, GSZ)]
                pre = []
                gctr = 0
                for ph in range(NPASS):
                    po = {}
                    for dc in range(NDC):
                        po[dc] = ps.tile([128, C1], F32, tag=f"y{dc}",
                                         name=f"po{dc}")
                    wts = []
                    for g0, nb in w2groups:
                        wt2 = w2pool.tile([128, GSZ * W], WDT, tag="w2",
                                          name="wt2")
                        nc.sync.dma_start(wt2[:, :nb * W],
                                          w2_t[ph][:, g0 * W:(g0 + nb) * W])
                        wts.append((wt2, g0, nb))
                        gctr += 1
                        if (next_slice is not None and gctr % 2 == 0
                                and len(pre) < 4):
                            gi = len(pre)
                            nb_n = groups_of(next_slice)[gi]
                            b0_n = sum(groups_of(next_slice)[:gi])
                            wt1n, wt3n = issue_w13(next_slice, gi, nb_n, b0_n)
                            pre.append((wt1n, wt3n, nb_n, b0_n))
                    # un-interleaved dc chains: dc0's drain (copy + output
                    # DMA) hides under dc1's matmul chain.
                    for dc in range(NDC):
                        for wt2, g0, nb in wts:
                            for sb in range(nb):
                                b = g0 + sb
                                lo = sb * W + dc * 128
                                nc.tensor.matmul(
                                    po[dc][:, :C],
                                    wt2[:, lo:lo + 128],
                                    h[:, b * C:(b + 1) * C],
                                    start=(b == 0), stop=(b == HB - 1))
                        ot = opool.tile([128, C1], F32, tag="ot")
                        nc.vector.tensor_copy(ot[:, :C], po[dc][:, :C])
                        nc.scalar.dma_start(
                            y_t[ph * W + dc * 128:ph * W + dc * 128 + 128, :],
                            ot[:, :C])
                return pre

            h0 = hpool.tile([128, HB * C1], WDT, tag="h0")
            h1 = hpool.tile([128, HB * C2], WDT, tag="h1")
            emit_phase1(0, h0, [])
            pre_b = emit_phase2(0, h0, next_slice=1)
            emit_phase1(1, h1, pre_b)
            emit_phase2(1, h1, next_slice=None)

    nc.compile()
    _BUILD_CACHE[key] = nc
    return nc


def _pack13(wh):
    """[2816, 2048] w1/w3 half -> phase-1 layout [128, HB*DB*128]:
    col = (b*DB + do)*128 + i_in, partition = d_in."""
    return np.ascontiguousarray(
        wh.reshape(HB, 128, DB, 128).transpose(3, 0, 2, 1)
    ).reshape(128, HB * DB * 128)


def _pack2(wh):
    """[2816, 2048] w2 half -> phase-2 layout [NPASS, 128, HB*W]:
    per pass, col = b*W + j, partition = i_in."""
    return np.ascontiguousarray(
        wh.reshape(HB, 128, NPASS, W).transpose(2, 1, 0, 3)
    ).reshape(NPASS, 128, HB * W)


def _packx(x_bf, tokens, C):
    """Gather token rows of x (bf16) and lay out as [128, DB*C]:
    col = do*C + c, partition = d_in."""
    xp = np.zeros((C, D), BF16)
    xp[:len(tokens)] = x_bf[tokens]
    return np.ascontiguousarray(
        xp.reshape(C, DB, 128).transpose(2, 1, 0)
    ).reshape(128, DB * C)


def _prepare(inputs):
    """Host routing + packing. Returns (nc, in_maps, scatter_info)."""
    x = np.asarray(inputs["x"])
    idx = np.asarray(inputs["expert_indices"])
    w1 = np.asarray(inputs["w1"])
    w2 = np.asarray(inputs["w2"])
    w3 = np.asarray(inputs["w3"])

    t_n, a_n = idx.shape

    # ---- dedup + routing ----
    tt = np.repeat(np.arange(t_n), a_n)
    ee = idx.reshape(-1).astype(np.int64)
    keys = tt * E + ee
    uniq = np.unique(keys)                        # sorted (t, e) pairs
    ue = uniq % E
    ut = uniq // E
    order = np.argsort(ue, kind="stable")         # grouped by expert
    counts = np.bincount(ue, minlength=E)
    starts = np.concatenate([[0], np.cumsum(counts)])
    # concat-layout row of each unique pair, and the gather map for scatter
    col = np.empty(len(uniq), np.int64)
    col[order] = np.arange(len(uniq)) - starts[ue[order]]
    concat_row = starts[ue] + col
    gather_rows = concat_row[np.searchsorted(uniq, keys)]   # [T*A]

    # ---- heavy/light pairing ----
    rank = np.argsort(-counts, kind="stable")
    pairs = [(int(rank[i]), int(rank[7 - i])) for i in range(4)]
    C1 = _pad4(counts[rank[0]])
    C2 = _pad4(counts[rank[4]])
    tokens_of = {
        int(e): ut[order[starts[e]:starts[e] + counts[e]]] for e in range(E)
    }

    nc = _build(C1, C2)

    x_bf = x.astype(BF16)
    w1_bf = {}
    in_maps = [dict() for _ in range(N_CORES)]
    for g, (he, le) in enumerate(pairs):
        xa = _packx(x_bf, tokens_of[he], C1)
        xb = _packx(x_bf, tokens_of[le], C2)
        for half in range(2):
            c = 2 * g + half
            r0, r1 = half * (I // 2), (half + 1) * (I // 2)
            in_maps[c]["xga"] = xa
            in_maps[c]["xgb"] = xb
            in_maps[c]["w1a"] = _pack13(w1[he][r0:r1].astype(BF16))
            in_maps[c]["w3a"] = _pack13(w3[he][r0:r1].astype(BF16))
            in_maps[c]["w2a"] = _pack2(w2[he][r0:r1].astype(BF16))
            in_maps[c]["w1b"] = _pack13(w1[le][r0:r1].astype(BF16))
            in_maps[c]["w3b"] = _pack13(w3[le][r0:r1].astype(BF16))
            in_maps[c]["w2b"] = _pack2(w2[le][r0:r1].astype(BF16))

    scatter_info = (t_n, a_n, pairs, counts, starts, gather_rows, len(uniq))
    return nc, in_maps, scatter_info


def _scatter(results, scatter_info):
    t_n, a_n, pairs, counts, starts, gather_rows, n_uniq = scatter_info
    yc = np.empty((n_uniq, D), np.float32)
    for g, (he, le) in enumerate(pairs):
        ya = results[2 * g]["yta"] + results[2 * g + 1]["yta"]   # [D, C1]
        yb = results[2 * g]["ytb"] + results[2 * g + 1]["ytb"]   # [D, C2]
        yc[starts[he]:starts[he] + counts[he]] = ya[:, :counts[he]].T
        yc[starts[le]:starts[le] + counts[le]] = yb[:, :counts[le]].T
    return yc[gather_rows].reshape(t_n, a_n, D)


def kernel(**inputs):
    from concourse.bass_utils import run_bass_kernel_spmd

    nc, in_maps, scatter_info = _prepare(inputs)
    res = run_bass_kernel_spmd(nc, in_maps, core_ids=list(range(N_CORES)))
    return _scatter(res.results, scatter_info)
